# revision 1
# baseline (speedup 1.0000x reference)
"""Trainium2 Bass kernel for nn_AdditiveAttention_28363964022887.

Strategy
--------
Data-parallel over batch B=32 across 8 NeuronCores (4 batches/core), weights
replicated, no collectives. Host gathers per-core outputs.

Per batch b the reference computes
    q = queries @ Wq;  k = keys @ Wk
    scores  = (q @ k.T) / sqrt(1024)              [2048, 50]
    scores2 = (W_o.T @ scores.T)                  [512, 2048]  (+ mask, axis q)
    attn    = softmax(scores2, axis=q)
    out     = attn @ (values @ Wv)                [512, 1024]

Algebraic reassociation cuts FLOPs ~3x vs the naive order:
    scoresT = (Wq^T/32 @ (keys @ Wk).T).T-chain: per batch only
              0.105 + 0.105 + 0.21 GF instead of projecting q (4.3 GF), and
    out     = ((attn @ values) @ Wv)  = 2.15 + 1.07 GF instead of 4.3 + 2.15.

The mask bias (0 or -1e6 per q) is folded into the W_o contraction as an
extra (51st) row: S2 = [W_o; 1]^T @ [scoresT; bias] so masking costs zero
elementwise work.  exp is evaluated twice, in both layouts:
    ET[q, v] (q on partitions) feeds the big contraction over q,
    E[v, q]  (v on partitions) yields Z (ACT accumulate) and the attn output.
The softmax normalisation 1/Z is applied at the very end (out has v on
partitions there) and on E for the attn output; exp(-1e6)=0 underflow
implements the mask exactly.  valid_len==0 batches (all-masked -> uniform
attention in the reference, 0/0 on device) are fixed up on the host.

Compute dtype bf16 (f32 PSUM accumulation), f32 outputs.
"""

import json

import ml_dtypes
import numpy as np

import concourse.bass as bass
import concourse.tile as tile
from concourse import mybir
from concourse.bass_utils import run_bass_kernel_spmd

N_CORES = 8
B, LQ, LK, D, HID, V = 32, 2048, 50, 1024, 1024, 512
BPC = B // N_CORES  # batches per core
NIT = D // 128      # 8 contraction tiles over D
NQT = LQ // 128     # 16 q tiles
NQC = LQ // 512     # 4 q chunks
NVC = V // 128      # 4 v chunks
NHC = HID // 512    # 2 h chunks
LKA = LK + 1        # augmented contraction (bias row)
BF16 = mybir.dt.bfloat16
F32 = mybir.dt.float32
MASK_VALUE = -1.0e6

# ---------------------------------------------------------------------------
# walrus in this container accepts only ONE sync-wait per instruction; Tile
# emits instructions with several.  Rewrite the BIR JSON at compile time:
# hoist all but the last wait onto single-wait EventSemaphore instructions.
# ---------------------------------------------------------------------------


def _split_multi_waits(bir_json: bytes) -> bytes:
    data = json.loads(bir_json)
    for fn in data.get("functions", []):
        for blk in fn.get("blocks", []):
            out = []
            for inst in blk.get("instructions", []):
                si = inst.get("sync_info")
                waits = (si or {}).get("on_wait") or []
                if len(waits) > 1:
                    for i, w in enumerate(waits[:-1]):
                        out.append(
                            {
                                "debug": inst.get("debug", 0),
                                "engine": inst["engine"],
                                "ins": [],
                                "name": f"{inst['name']}_wsplit{i}",
                                "opcode": "EventSemaphore",
                                "outs": [],
                                "sync_info": {"on_update": [], "on_wait": [w]},
                            }
                        )
                    si["on_wait"] = [waits[-1]]
                out.append(inst)
            blk["instructions"] = out
    return json.dumps(data).encode()


_waitfix_installed = False


def _install_waitfix():
    global _waitfix_installed
    if _waitfix_installed:
        return
    _waitfix_installed = True
    import concourse.bass2jax as bass2jax
    import concourse.bass_utils as bass_utils

    orig = bass_utils.compile_bir_kernel

    def wrapped(bir_json, tmpdir, neff_name="file.neff"):
        return orig(_split_multi_waits(bir_json), tmpdir, neff_name=neff_name)

    bass_utils.compile_bir_kernel = wrapped
    bass2jax.compile_bir_kernel = wrapped


# ---------------------------------------------------------------------------
# device kernel
# ---------------------------------------------------------------------------


def build_nc(bpc: int = BPC) -> bass.Bass:
    _install_waitfix()
    nc = bass.Bass("TRN2", target_bir_lowering=False, debug=False)

    qt_d = nc.dram_tensor("qt", [bpc, D, LQ], BF16, kind="ExternalInput").ap()
    kt_d = nc.dram_tensor("kt", [bpc, D, LK], BF16, kind="ExternalInput").ap()
    vals_d = nc.dram_tensor("vals", [bpc, LQ, HID], BF16, kind="ExternalInput").ap()
    bias_d = nc.dram_tensor("bias", [bpc, LQ], BF16, kind="ExternalInput").ap()
    wqt_d = nc.dram_tensor("wqt", [D, D], BF16, kind="ExternalInput").ap()
    wk_d = nc.dram_tensor("wk", [D, D], BF16, kind="ExternalInput").ap()
    wv_d = nc.dram_tensor("wv", [D, HID], BF16, kind="ExternalInput").ap()
    woa_d = nc.dram_tensor("woa", [LKA, V], BF16, kind="ExternalInput").ap()

    out_d = nc.dram_tensor("out", [bpc, V, HID], F32, kind="ExternalOutput").ap()
    attn_d = nc.dram_tensor("attn", [bpc, V, LQ], BF16, kind="ExternalOutput").ap()

    Exp = mybir.ActivationFunctionType.Exp

    with tile.TileContext(nc) as tc:
        with (
            tc.tile_pool(name="weights", bufs=1) as wpool,
            tc.tile_pool(name="qtp", bufs=NIT + 1) as qtp,
            tc.tile_pool(name="valp", bufs=NQT + 1) as valp,
            tc.tile_pool(name="small", bufs=2) as smallp,
            tc.tile_pool(name="sctp", bufs=2) as sctp,
            tc.tile_pool(name="etp", bufs=NQT + 2) as etp,
            tc.tile_pool(name="ep", bufs=NVC + 1) as epool,
            tc.tile_pool(name="oatp", bufs=2) as oatp,
            tc.tile_pool(name="outp", bufs=4) as outp,
            tc.tile_pool(name="zp", bufs=2) as zpool,
            tc.tile_pool(name="ps_small", bufs=2, space="PSUM") as ps_small,
            tc.tile_pool(name="ps_s2", bufs=2, space="PSUM") as ps_s2,
            tc.tile_pool(name="ps_big", bufs=3, space="PSUM") as ps_big,
        ):
            # weights, resident for the whole kernel
            wk_sb = wpool.tile([128, NIT, D], BF16)  # [i_p, it, h]
            nc.sync.dma_start(out=wk_sb[:], in_=wk_d.rearrange("(t p) h -> p t h", p=128))
            wqt_sb = wpool.tile([128, NIT, D], BF16)  # [h_p, ht, i]
            nc.sync.dma_start(out=wqt_sb[:], in_=wqt_d.rearrange("(t p) i -> p t i", p=128))
            wv_sb = wpool.tile([128, NIT, HID], BF16)  # [i_p, it, h]
            nc.sync.dma_start(out=wv_sb[:], in_=wv_d.rearrange("(t p) h -> p t h", p=128))
            woa_sb = wpool.tile([LKA, V], BF16)
            nc.sync.dma_start(out=woa_sb[:], in_=woa_d[:])

            for b in range(bpc):
                # keys^T for this batch: [i_p, it, k']
                keyst = smallp.tile([128, NIT, LK], BF16, tag="keyst")
                nc.sync.dma_start(
                    out=keyst[:], in_=kt_d[b].rearrange("(t p) k -> p t k", p=128)
                )

                # kT[h, k'] = sum_i Wk[i, h] keysT[i, k']
                kt_sb = smallp.tile([128, NIT, LK], BF16, tag="kt")  # [h_p, ht, k']
                for ht in range(NIT):
                    ps = ps_small.tile([128, 512], F32, tag="small_ps")
                    pk = ps[:, :LK]
                    for it in range(NIT):
                        nc.tensor.matmul(
                            pk,
                            lhsT=wk_sb[:, it, ht * 128 : (ht + 1) * 128],
                            rhs=keyst[:, it, :],
                            start=(it == 0),
                            stop=(it == NIT - 1),
                        )
                    nc.scalar.copy(out=kt_sb[:, ht, :], in_=pk)

                # M[i, k'] = sum_h (Wq[i, h]/32) kT[h, k']
                m_sb = smallp.tile([128, NIT, LK], BF16, tag="m")  # [i_p, ic, k']
                for ic in range(NIT):
                    ps = ps_small.tile([128, 512], F32, tag="small_ps")
                    pm = ps[:, :LK]
                    for ht in range(NIT):
                        nc.tensor.matmul(
                            pm,
                            lhsT=wqt_sb[:, ht, ic * 128 : (ic + 1) * 128],
                            rhs=kt_sb[:, ht, :],
                            start=(ht == 0),
                            stop=(ht == NIT - 1),
                        )
                    nc.scalar.copy(out=m_sb[:, ic, :], in_=pm)

                # scoresT_aug[0:50, q] = sum_i M[i, k'] qT[i, q]; row 50 = bias
                sct = sctp.tile([LKA, LQ], BF16, tag="sct")
                nc.sync.dma_start(out=sct[LK:LKA, :], in_=bias_d[b : b + 1, :])
                for qh in range(2):  # halves of q so the qt pool stays small
                    qts = []
                    for it in range(NIT):
                        q = qtp.tile([128, 1024], BF16, tag="qt")
                        nc.sync.dma_start(
                            out=q[:],
                            in_=qt_d[b, it * 128 : (it + 1) * 128, qh * 1024 : (qh + 1) * 1024],
                        )
                        qts.append(q)
                    for qcl in range(2):
                        qc = qh * 2 + qcl
                        ps = ps_small.tile([128, 512], F32, tag="small_ps")
                        pt = ps[:LK, :]
                        for it in range(NIT):
                            nc.tensor.matmul(
                                pt,
                                lhsT=m_sb[:, it, :],
                                rhs=qts[it][:, qcl * 512 : (qcl + 1) * 512],
                                start=(it == 0),
                                stop=(it == NIT - 1),
                            )
                        nc.scalar.copy(out=sct[:LK, qc * 512 : (qc + 1) * 512], in_=pt)

                # ET[q, v] = exp(scoresT_aug^T @ [W_o; 1] ) , q on partitions
                ets = []
                for qt_i in range(NQT):
                    ps = ps_s2.tile([128, 512], F32, tag="s2")
                    nc.tensor.matmul(
                        ps,
                        lhsT=sct[:, qt_i * 128 : (qt_i + 1) * 128],
                        rhs=woa_sb[:],
                        start=True,
                        stop=True,
                    )
                    et = etp.tile([128, 512], BF16, tag="et")
                    nc.scalar.activation(out=et[:], in_=ps, func=Exp)
                    ets.append(et)

                # E[v, q] = exp(S2), v on partitions; ACT accumulates Z per row
                zp = zpool.tile([128, NVC, NQC], F32, tag="zpart")
                es = []
                for vc in range(NVC):
                    e = epool.tile([128, LQ], BF16, tag="e")
                    for qc in range(NQC):
                        ps = ps_s2.tile([128, 512], F32, tag="s2")
                        nc.tensor.matmul(
                            ps,
                            lhsT=woa_sb[:, vc * 128 : (vc + 1) * 128],
                            rhs=sct[:, qc * 512 : (qc + 1) * 512],
                            start=True,
                            stop=True,
                        )
                        nc.scalar.activation(
                            out=e[:, qc * 512 : (qc + 1) * 512],
                            in_=ps,
                            func=Exp,
                            accum_out=zp[:, vc, qc : qc + 1],
                        )
                    es.append(e)
                z = zpool.tile([128, NVC], F32, tag="z")
                nc.vector.reduce_sum(out=z[:], in_=zp[:], axis=mybir.AxisListType.X)
                rz = zpool.tile([128, NVC], F32, tag="rz")
                nc.vector.reciprocal(out=rz[:], in_=z[:])

                # attn = E / Z  (in place), then store
                for vc in range(NVC):
                    nc.vector.tensor_scalar_mul(es[vc][:], es[vc][:], rz[:, vc : vc + 1])
                    nc.gpsimd.dma_start(
                        out=attn_d[b, vc * 128 : (vc + 1) * 128, :], in_=es[vc][:]
                    )

                # values tiles, q on partitions
                vts = []
                for qt_i in range(NQT):
                    v = valp.tile([128, HID], BF16, tag="vals")
                    nc.sync.dma_start(
                        out=v[:], in_=vals_d[b, qt_i * 128 : (qt_i + 1) * 128, :]
                    )
                    vts.append(v)

                # oat[i, v] = sum_q values[q, i] ET[q, v]   (unnormalised)
                oat = oatp.tile([128, NIT, V], BF16, tag="oat")  # [i_p, it, v]
                for it in range(NIT):
                    ps = ps_big.tile([128, 512], F32, tag="big")
                    for qt_i in range(NQT):
                        nc.tensor.matmul(
                            ps,
                            lhsT=vts[qt_i][:, it * 128 : (it + 1) * 128],
                            rhs=ets[qt_i][:],
                            start=(qt_i == 0),
                            stop=(qt_i == NQT - 1),
                        )
                    nc.vector.tensor_copy(out=oat[:, it, :], in_=ps)

                # out[v, h] = (sum_i oat[i, v] Wv[i, h]) / Z_v
                for vc in range(NVC):
                    for hc in range(NHC):
                        ps = ps_big.tile([128, 512], F32, tag="big")
                        for it in range(NIT):
                            nc.tensor.matmul(
                                ps,
                                lhsT=oat[:, it, vc * 128 : (vc + 1) * 128],
                                rhs=wv_sb[:, it, hc * 512 : (hc + 1) * 512],
                                start=(it == 0),
                                stop=(it == NIT - 1),
                            )
                        o = outp.tile([128, 512], F32, tag="o")
                        nc.vector.tensor_scalar_mul(o[:], ps, rz[:, vc : vc + 1])
                        nc.gpsimd.dma_start(
                            out=out_d[b, vc * 128 : (vc + 1) * 128, hc * 512 : (hc + 1) * 512],
                            in_=o[:],
                        )
    return nc


# ---------------------------------------------------------------------------
# host wrapper
# ---------------------------------------------------------------------------

_nc_cache: dict[int, bass.Bass] = {}


def _get_nc(bpc: int = BPC) -> bass.Bass:
    if bpc not in _nc_cache:
        _nc_cache[bpc] = build_nc(bpc)
    return _nc_cache[bpc]


def make_in_maps(queries, keys, values, valid_lens, W_q, W_k, W_v, W_o):
    bf = ml_dtypes.bfloat16
    queries = np.asarray(queries, np.float32)
    keys = np.asarray(keys, np.float32)
    values = np.asarray(values, np.float32)
    vl = np.asarray(valid_lens).astype(np.int64)
    W_q = np.asarray(W_q, np.float32)
    W_k = np.asarray(W_k, np.float32)
    W_v = np.asarray(W_v, np.float32)
    W_o = np.asarray(W_o, np.float32)

    qt = np.ascontiguousarray(queries.astype(bf).transpose(0, 2, 1))  # [B, D, LQ]
    kt = np.ascontiguousarray(keys.astype(bf).transpose(0, 2, 1))  # [B, D, LK]
    vals = values.astype(bf)
    wqt = np.ascontiguousarray((W_q / np.sqrt(D)).T).astype(bf)
    wk = W_k.astype(bf)
    wv = W_v.astype(bf)
    woa = np.concatenate([W_o, np.ones((1, V), np.float32)], 0).astype(bf)
    bias = np.where(
        np.arange(LQ)[None, :] < vl[:, None], np.float32(0.0), np.float32(MASK_VALUE)
    ).astype(bf)

    in_maps = []
    for c in range(N_CORES):
        s = slice(c * BPC, (c + 1) * BPC)
        in_maps.append(
            {
                "qt": qt[s],
                "kt": kt[s],
                "vals": vals[s],
                "bias": bias[s],
                "wqt": wqt,
                "wk": wk,
                "wv": wv,
                "woa": woa,
            }
        )
    return in_maps, vl


def _fixup_all_masked(out, attn, vl, values, W_v):
    """valid_len==0 rows: reference softmax over all -1e6 is uniform."""
    zero = np.nonzero(vl == 0)[0]
    if zero.size:
        attn[zero] = np.float32(1.0 / LQ)
        v_proj = np.asarray(values, np.float32)[zero] @ np.asarray(W_v, np.float32)
        mean = v_proj.mean(axis=1)  # [n, HID]
        out[zero] = np.broadcast_to(mean[:, None, :], (zero.size, V, HID))
    return out, attn


def kernel(queries, keys, values, valid_lens, W_q, W_k, W_v, W_o):
    in_maps, vl = make_in_maps(
        queries, keys, values, valid_lens, W_q, W_k, W_v, W_o
    )
    nc = _get_nc()
    res = run_bass_kernel_spmd(nc, in_maps, core_ids=list(range(N_CORES)))
    out = np.concatenate([r["out"] for r in res.results], 0)
    attn = np.concatenate(
        [r["attn"].astype(np.float32) for r in res.results], 0
    )
    out, attn = _fixup_all_masked(out, attn, vl, values, W_v)
    return out, attn
